# revision 1
# baseline (speedup 1.0000x reference)
"""BiLSTM+CRF loss kernel for Trainium2 (8 NeuronCores, data-parallel over batch).

Self-contained: hardcodes shapes B=64, T=2048, V=4096, E=H=128, C=8.

Per-core algorithm (batch-local BL=8, both LSTM directions):
  - Host precomputes W' = emb @ Wih.T + b  (weights-only transform), tanh-form
    scaled (sigmoid(x) = (tanh(x/2)+1)/2), packed as bf16 gather tables.
  - GPSIMD ap_gather pulls per-(t,b) input projections straight from SBUF
    tables (fused embedding lookup + input projection, no transposes).
  - Recurrence (gate-partition layout [128=H, gates x batch]):
      psum P = gathered xproj (via identity matmul) + 4 accumulating bf16
      matmuls Whh~_c @ H2;  t = tanh(P) (one ACT op, all gates, both dirs);
      3 scalar_tensor_tensor DVE ops update C2 = 2c and H2 = 2h.
  - logits via H2-as-lhsT matmuls -> [t*b, C] layout in DRAM.
  - CRF log-partition as a log-semiring binary product tree over 2048 leaf
    matrices (exact, max-shifted), exp/ln passes split per level to avoid
    ACT table thrash.
"""
import os
import sys
import numpy as np
import ml_dtypes

sys.path.insert(0, "/opt/trn_rl_repo")

from contextlib import ExitStack

import concourse.bass as bass
import concourse.tile as tile
from concourse import bacc, mybir
from concourse import bass_utils

B, T, V, E, H, C = 64, 2048, 4096, 128, 128, 8
NCORE = 8
BL = B // NCORE
GATE_PERM = [0, 1, 3, 2]          # device gate order [i,f,o,g] from ref [i,f,g,o]
GATE_SCALE = [0.5, 0.5, 0.5, 1.0]
W = 32                            # gather window (steps per ring refill)

F32 = mybir.dt.float32
BF16 = mybir.dt.bfloat16
I16 = mybir.dt.int16
AF = mybir.ActivationFunctionType
ALU = mybir.AluOpType


def _bf(a):
    return np.asarray(a, np.float32).astype(ml_dtypes.bfloat16)


# ---------------------------------------------------------------- host prep

def _reorder_gates(w):
    ch = np.split(np.asarray(w, np.float32), 4, axis=0)
    return [ch[p] for p in GATE_PERM]


def host_prep(inputs, T_=T):
    """Build device tensors. Returns (shared: dict, per_core_idx: list)."""
    x = np.asarray(inputs["x"])
    emb = np.asarray(inputs["emb"], np.float32)
    fc_w = np.asarray(inputs["fc_w"], np.float32)
    fc_b = np.asarray(inputs["fc_b"], np.float32)
    trans = np.asarray(inputs["trans"], np.float32)
    start = np.asarray(inputs["start"], np.float32)
    end = np.asarray(inputs["end"], np.float32)

    # gather tables: tbl[d, pair, p, v, e]; value = s_c*(emb[v]@Wih_c.T + b_c)[p]
    # with c = 2*pair + e, gate order [i,f,o,g]
    tbl = np.zeros((2, 2, H, V, 2), np.float32)
    whh = np.zeros((H, 2 * 4 * H), np.float32)   # cols (d*4+c)*128+m, lhsT layout
    for d, (wih_k, whh_k, b_k) in enumerate(
        [("Wih_f", "Whh_f", "b_f"), ("Wih_b", "Whh_b", "b_b")]
    ):
        Wc = _reorder_gates(inputs[wih_k])
        bc = _reorder_gates(np.asarray(inputs[b_k], np.float32)[:, None])
        Hc = _reorder_gates(inputs[whh_k])
        for c in range(4):
            s = GATE_SCALE[c]
            tbl[d, c // 2, :, :, c % 2] = (s * (emb @ Wc[c].T + bc[c].T)).T
            whh[:, (d * 4 + c) * H:(d * 4 + c + 1) * H] = ((s / 2.0) * Hc[c]).T

    # fc rhs: fcw[k, d*8+j] = 0.5*fc_w[j, d*128+k]
    fcw = np.zeros((H, 16), np.float32)
    fcw[:, 0:8] = 0.5 * fc_w[:, :H].T
    fcw[:, 8:16] = 0.5 * fc_w[:, H:].T

    # CRF pair-product constants: tt2[g][p, (i,k,j)]
    i_, k_, j_ = np.meshgrid(np.arange(C), np.arange(C), np.arange(C), indexing="ij")
    plain = (trans[i_, j_] + trans[j_, k_]).reshape(-1).astype(np.float32)  # [512]
    # special (leaf t=0): value = start[j] + trans[j,k], independent of i
    spec = (start[j_] + trans[j_, k_]).reshape(-1).astype(np.float32)
    tt2 = np.zeros((2, 128, 512), np.float32)
    tt2[0, :, :] = plain[None, :]
    tt2[1, :, :] = plain[None, :]
    tt2[1, 0:8, :] = spec[None, :]    # partitions 0..7 hold node n=0 in group 0

    endrep = np.broadcast_to(end[None, :], (8, 8)).copy().astype(np.float32)

    shared = {
        "tbl": _bf(tbl.transpose(2, 0, 1, 3, 4).reshape(H, -1)),
        "whh": _bf(whh),
        "ident": _bf(np.eye(H, dtype=np.float32)),
        "fcw": _bf(fcw),
        "ones1": np.ones((1, H), np.float32),
        "fcb1": fc_b.reshape(1, C).astype(np.float32),
        "tt2": tt2,
        "endrep": endrep,
    }

    per_core = []
    nidx = T_ * BL // 16
    for core in range(NCORE):
        xc = x[core * BL:(core + 1) * BL, :]   # [BL, T]
        idx = np.zeros((128, 2 * nidx), np.int16)
        for d in range(2):
            ind = np.empty(T_ * BL, np.int64)
            for s in range(T_):
                t = s if d == 0 else (T_ - 1 - s)
                ind[s * BL:(s + 1) * BL] = xc[:, t]
            wrap = np.zeros((16, nidx), np.int16)
            for n in range(T_ * BL):
                wrap[n % 16, n // 16] = ind[n]
            idx[:, d * nidx:(d + 1) * nidx] = np.tile(wrap, (8, 1))
        per_core.append({"idx": idx})
    return shared, per_core


# ---------------------------------------------------------------- device build

def _crf_product_phase_a(nc, ctmp, in0_ap, in1_ap, rmx_out, ssum_out, pcount):
    """tmp = in0 + in1 (APs already [p,(g?),i,k,j]); rmax; sub; exp; sum."""
    shape = tuple(in0_ap.shape)
    nfree = int(np.prod(shape[1:]))
    tmp = ctmp.tile([pcount, nfree], F32, tag="tmpA")
    tmp4 = tmp[:].rearrange("p (i k j) -> p i k j", i=shape[1], k=shape[2])
    nc.vector.tensor_tensor(tmp4, in0_ap, in1_ap, ALU.add)
    nc.vector.tensor_reduce(rmx_out, tmp4, axis=mybir.AxisListType.X, op=ALU.max)
    rb = rmx_out.rearrange("p (i k) -> p i k", i=shape[1]).unsqueeze(3).broadcast_to(shape)
    tmp2 = ctmp.tile([pcount, nfree], F32, tag="tmpB")
    tmp24 = tmp2[:].rearrange("p (i k j) -> p i k j", i=shape[1], k=shape[2])
    nc.vector.tensor_sub(tmp24, tmp4, rb)
    nc.scalar.activation(tmp[:], tmp2[:], AF.Exp)
    nc.vector.tensor_reduce(ssum_out, tmp4, axis=mybir.AxisListType.X, op=ALU.add)


def build_module(T_=T, n_cores=NCORE):
    S = min(128, T_)                     # h2 staging block (steps)
    nidx = T_ * BL // 16
    NW = T_ * BL // 128                  # fc windows
    NLEAF = T_                           # tree leaves (power of 2)
    NPAIR = NLEAF // 2

    nc = bacc.Bacc("TRN2", target_bir_lowering=False, debug=False,
                   enable_asserts=False, num_devices=n_cores)

    tbl_d = nc.dram_tensor("tbl", [H, 2 * 2 * V * 2], BF16, kind="ExternalInput").ap()
    whh_d = nc.dram_tensor("whh", [H, 8 * H], BF16, kind="ExternalInput").ap()
    ident_d = nc.dram_tensor("ident", [H, H], BF16, kind="ExternalInput").ap()
    fcw_d = nc.dram_tensor("fcw", [H, 16], BF16, kind="ExternalInput").ap()
    ones1_d = nc.dram_tensor("ones1", [1, H], F32, kind="ExternalInput").ap()
    fcb1_d = nc.dram_tensor("fcb1", [1, C], F32, kind="ExternalInput").ap()
    tt2_d = nc.dram_tensor("tt2", [2, 128, 512], F32, kind="ExternalInput").ap()
    endrep_d = nc.dram_tensor("endrep", [8, 8], F32, kind="ExternalInput").ap()
    idx_d = nc.dram_tensor("idx", [128, 2 * nidx], I16, kind="ExternalInput").ap()
    out_d = nc.dram_tensor("out", [8, 1], F32, kind="ExternalOutput").ap()

    h2f_d = nc.dram_tensor("h2f_i", [H, T_ * BL], BF16).ap()
    h2b_d = nc.dram_tensor("h2b_i", [H, T_ * BL], BF16).ap()
    logits_d = nc.dram_tensor("logits_i", [T_ * BL, C], F32).ap()

    with tile.TileContext(nc) as tc, ExitStack() as ctx:
        psum = ctx.enter_context(tc.tile_pool(name="psum", bufs=2, space="PSUM"))
        persist = ctx.enter_context(tc.tile_pool(name="persist", bufs=1))
        ringp = ctx.enter_context(tc.tile_pool(name="ringp", bufs=1))
        scr = ctx.enter_context(tc.tile_pool(name="scr", bufs=2))
        stagep = ctx.enter_context(tc.tile_pool(name="stagep", bufs=2))

        # ---- load persistent tensors
        tbl = persist.tile([H, 2 * 2 * V * 2], BF16)
        nc.sync.dma_start(tbl[:], tbl_d[:])
        whh = persist.tile([H, 8 * H], BF16)
        nc.sync.dma_start(whh[:], whh_d[:])
        ident = persist.tile([H, H], BF16)
        nc.sync.dma_start(ident[:], ident_d[:])
        idxt = persist.tile([128, 2 * nidx], I16)
        nc.sync.dma_start(idxt[:], idx_d[:])
        fcw = persist.tile([H, 16], BF16)
        nc.sync.dma_start(fcw[:], fcw_d[:])
        ones1 = persist.tile([1, H], F32)
        nc.sync.dma_start(ones1[:], ones1_d[:])
        fcb1 = persist.tile([1, C], F32)
        nc.sync.dma_start(fcb1[:], fcb1_d[:])

        M = persist.tile([128, 80], F32)
        nc.vector.memset(M[:, 32:40], 0.0)
        nc.vector.memset(M[:, 72:80], 0.0)
        h2init = persist.tile([128, 16], BF16)
        nc.vector.memset(h2init[:], 0.0)

        ring = [ringp.tile([128, 2 * 2 * W * BL * 2], BF16, tag=f"ring{p}",
                           name=f"ring{p}")
                for p in range(2)]
        tbl5 = tbl[:].rearrange("p (d q v e) -> p d q v e", d=2, q=2, e=2)

        # ---- recurrence
        h2prev = {0: h2init[:, 0:8], 1: h2init[:, 8:16]}
        stf = stb = None
        for s in range(T_):
            if s % W == 0:
                rt = ring[(s // W) % 2]
                r5 = rt[:].rearrange("p (d q n e) -> p d q n e", d=2, q=2, e=2)
                for d in range(2):
                    for q in range(2):
                        nc.gpsimd.ap_gather(
                            r5[:, d, q, :, :],
                            tbl5[:, d, q, :, :],
                            idxt[:, d * nidx + s * BL // 16:
                                 d * nidx + (s + W) * BL // 16],
                            channels=128, num_elems=V, d=2, num_idxs=W * BL,
                        )
            if s % S == 0:
                stf = stagep.tile([128, S * BL], BF16, tag="stf")
                stb = stagep.tile([128, S * BL], BF16, tag="stb")

            P = psum.tile([128, 8 * BL], F32, tag="P")
            rhs = (ring[(s // W) % 2][:]
                   .rearrange("p (d q n e) -> p d q e n", d=2, q=2, e=2)
                   [:, :, :, :, (s % W) * BL:(s % W + 1) * BL])
            nc.tensor.matmul(P[:], ident[:], rhs, start=True, stop=False,
                             skip_group_check=True)
            for d in range(2):
                for cq in range(4):
                    col = d * 4 * BL + cq * BL
                    nc.tensor.matmul(
                        P[:, col:col + BL],
                        whh[:, (d * 4 + cq) * H:(d * 4 + cq + 1) * H],
                        h2prev[d], start=False, stop=(cq == 3),
                        skip_group_check=True)

            M3 = M[:].rearrange("p (d t) -> p d t", d=2)
            P3 = P[:].rearrange("p (d t) -> p d t", d=2)
            nc.scalar.activation(M3[:, :, 0:4 * BL], P3[:], AF.Tanh)
            X = scr.tile([128, 4 * BL], F32, tag="X")
            X3 = X[:].rearrange("p (d t) -> p d t", d=2)
            nc.vector.scalar_tensor_tensor(
                X3, M3[:, :, 0:2 * BL], 1.0, M3[:, :, 3 * BL:5 * BL],
                ALU.add, ALU.mult)
            nc.vector.scalar_tensor_tensor(
                M3[:, :, 4 * BL:5 * BL], X3[:, :, BL:2 * BL], 0.5,
                X3[:, :, 0:BL], ALU.mult, ALU.add)
            th = scr.tile([128, 2 * BL], F32, tag="th")
            th3 = th[:].rearrange("p (d t) -> p d t", d=2)
            nc.scalar.activation(th3, M3[:, :, 4 * BL:5 * BL], AF.Tanh, scale=0.5)
            fs = (s % S) * BL
            bs_ = (S - 1 - (s % S)) * BL
            nc.vector.scalar_tensor_tensor(
                stf[:, fs:fs + BL], M[:, 2 * BL:3 * BL], 1.0, th[:, 0:BL],
                ALU.add, ALU.mult)
            nc.vector.scalar_tensor_tensor(
                stb[:, bs_:bs_ + BL], M[:, 5 * BL + 2 * BL:5 * BL + 3 * BL],
                1.0, th[:, BL:2 * BL], ALU.add, ALU.mult)
            h2prev = {0: stf[:, fs:fs + BL], 1: stb[:, bs_:bs_ + BL]}
            if s % S == S - 1:
                blk = s // S
                nc.sync.dma_start(h2f_d[:, blk * S * BL:(blk + 1) * S * BL], stf[:])
                tbase = (T_ - S * (blk + 1)) * BL
                nc.sync.dma_start(h2b_d[:, tbase:tbase + S * BL], stb[:])

        # ---- fc -> logits (DRAM, rows t*BL+b)
        fcpool = ctx.enter_context(tc.tile_pool(name="fcp", bufs=3))
        lstagep = ctx.enter_context(tc.tile_pool(name="lst", bufs=2))
        LG = min(8, NW)
        lst = None
        for w in range(NW):
            hf = fcpool.tile([128, 128], BF16, tag="hf")
            nc.sync.dma_start(hf[:], h2f_d[:, w * 128:(w + 1) * 128])
            hb = fcpool.tile([128, 128], BF16, tag="hb")
            nc.sync.dma_start(hb[:], h2b_d[:, w * 128:(w + 1) * 128])
            PL = psum.tile([128, C], F32, tag="PL")
            nc.tensor.matmul(PL[:], hf[:], fcw[:, 0:8], start=True, stop=False)
            nc.tensor.matmul(PL[:], hb[:], fcw[:, 8:16], start=False, stop=False)
            nc.tensor.matmul(PL[:], ones1[:], fcb1[:], start=False, stop=True)
            if w % LG == 0:
                lst = lstagep.tile([128, LG * 8], F32, tag="lstg")
            nc.scalar.copy(lst[:, (w % LG) * 8:(w % LG) * 8 + 8], PL[:])
            if w % LG == LG - 1:
                oap = (logits_d[:].rearrange("(w p) j -> p w j", p=128)
                       [:, (w // LG) * LG:(w // LG + 1) * LG, :])
                nc.sync.dma_start(oap, lst[:])

        # ---- CRF tree
        crf = ctx.enter_context(tc.tile_pool(name="crf", bufs=1))
        ctmp = ctx.enter_context(tc.tile_pool(name="ctmp", bufs=2))

        tt2p = crf.tile([128, 512], F32)
        nc.sync.dma_start(tt2p[:], tt2_d[0])
        tt2s = crf.tile([128, 512], F32)
        nc.sync.dma_start(tt2s[:], tt2_d[1])
        endt = crf.tile([8, 8], F32)
        nc.sync.dma_start(endt[:], endrep_d[:])

        G0 = max(1, NPAIR // 16)          # level-0 groups of <=128 instances
        # leaf emissions, level-0 layout
        LA = crf.tile([128, G0 * 8], F32)
        LB = crf.tile([128, G0 * 8], F32)
        hi_n = max(1, NPAIR // 16)
        lg5 = logits_d[:].rearrange("(hi g s lo) j -> s hi lo g j",
                                    hi=16, g=hi_n, s=2, lo=8)
        for hi in range(16):
            nc.sync.dma_start(LA[hi * 8:(hi + 1) * 8, :], lg5[0][hi])
            nc.sync.dma_start(LB[hi * 8:(hi + 1) * 8, :], lg5[1][hi])

        rmxa = crf.tile([128, G0 * 64], F32)
        ssma = crf.tile([128, G0 * 64], F32)
        lnt = crf.tile([128, G0 * 64], F32)
        arrs = {}
        arrs[1] = crf.tile([128, G0 * 64], F32, name="arr1")

        # level 0: P1 = LSE_j(tt2 + A[j]) + B[k]
        for g in range(G0):
            t4 = (tt2s if g == 0 else tt2p)[:].rearrange(
                "p (i k j) -> p i k j", i=8, k=8)
            a_ap = (LA[:, g * 8:(g + 1) * 8].unsqueeze(1).unsqueeze(1)
                    .broadcast_to((128, 8, 8, 8)))
            _crf_product_phase_a(nc, ctmp, t4, a_ap,
                                 rmxa[:, g * 64:(g + 1) * 64],
                                 ssma[:, g * 64:(g + 1) * 64], 128)
        nc.scalar.activation(lnt[:], ssma[:, 0:G0 * 64], AF.Ln)
        nc.vector.tensor_add(lnt[:], lnt[:], rmxa[:, 0:G0 * 64])
        b_ap = (LB[:].rearrange("p (g k) -> p g k", g=G0).unsqueeze(2)
                .broadcast_to((128, G0, 8, 8)))
        l4 = lnt[:].rearrange("p (g i k) -> p g i k", g=G0, i=8)
        o4 = arrs[1][:].rearrange("p (g i k) -> p g i k", g=G0, i=8)
        nc.vector.tensor_tensor(o4, l4, b_ap, ALU.add)

        # levels 1.. while >=16 nodes: high-bits mapping, groups halve
        lvl = 1
        N = NPAIR               # nodes in arrs[lvl]
        while N > 16:
            Gn = (N // 2) // 16
            arrs[lvl + 1] = crf.tile([128, max(Gn, 1) * 64], F32,
                                     tag=f"arr{lvl+1}", name=f"arr{lvl+1}")
            for g in range(Gn):
                A = arrs[lvl][:, (2 * g) * 64:(2 * g + 1) * 64]
                Bv = arrs[lvl][:, (2 * g + 1) * 64:(2 * g + 2) * 64]
                a_ap = (A.rearrange("p (i j) -> p i j", i=8).unsqueeze(2)
                        .broadcast_to((128, 8, 8, 8)))
                b_ap = (Bv.rearrange("p (j k) -> p k j", j=8).unsqueeze(1)
                        .broadcast_to((128, 8, 8, 8)))
                _crf_product_phase_a(nc, ctmp, a_ap, b_ap,
                                     rmxa[:, g * 64:(g + 1) * 64],
                                     ssma[:, g * 64:(g + 1) * 64], 128)
            nc.scalar.activation(lnt[:, 0:Gn * 64], ssma[:, 0:Gn * 64], AF.Ln)
            nc.vector.tensor_add(arrs[lvl + 1][:, 0:Gn * 64], lnt[:, 0:Gn * 64],
                                 rmxa[:, 0:Gn * 64])
            lvl += 1
            N //= 2

        # top levels: N=16 -> 1, de-interleave partitions via a DRAM bounce
        dscr_d = nc.dram_tensor("deint_i", [128, 64], F32).ap()
        cur = arrs[lvl]          # [128, 64], instance p = n*8+b, N nodes
        while N > 1:
            pc = N * 8           # current instances
            half = pc // 2
            nc.sync.dma_start(dscr_d[0:pc, :], cur[:])
            asp = dscr_d[0:pc, :].rearrange("(n s b) f -> s n b f",
                                            n=N // 2, s=2, b=8)
            at = crf.tile([half, 64], F32, tag=f"ta{N}", name=f"ta{N}")
            bt = crf.tile([half, 64], F32, tag=f"tb{N}", name=f"tb{N}")
            nc.sync.dma_start(at[:], asp[0])
            nc.sync.dma_start(bt[:], asp[1])
            nxt = crf.tile([half, 64], F32, tag=f"tn{N}")
            a_ap = (at[:].rearrange("p (i j) -> p i j", i=8).unsqueeze(2)
                    .broadcast_to((half, 8, 8, 8)))
            b_ap = (bt[:].rearrange("p (j k) -> p k j", j=8).unsqueeze(1)
                    .broadcast_to((half, 8, 8, 8)))
            rm = ctmp.tile([half, 64], F32, tag="rmtop")
            sm = ctmp.tile([half, 64], F32, tag="smtop")
            _crf_product_phase_a(nc, ctmp, a_ap, b_ap, rm[:], sm[:], half)
            ln_ = ctmp.tile([half, 64], F32, tag="lntop")
            nc.scalar.activation(ln_[:], sm[:], AF.Ln)
            nc.vector.tensor_add(nxt[:], ln_[:], rm[:])
            cur = nxt
            N //= 2

        # final: logZ_b = LSE_k(root[b, (0,k)] + end[k])
        z = ctmp.tile([8, 8], F32, tag="z")
        nc.vector.tensor_add(z[:], cur[:, 0:8], endt[:])
        zm = ctmp.tile([8, 1], F32, tag="zm")
        nc.vector.tensor_reduce(zm[:], z[:], axis=mybir.AxisListType.X, op=ALU.max)
        z2 = ctmp.tile([8, 8], F32, tag="z2")
        nc.vector.tensor_sub(z2[:], z[:], zm[:].broadcast_to((8, 8)))
        nc.scalar.activation(z2[:], z2[:], AF.Exp)
        zs = ctmp.tile([8, 1], F32, tag="zs")
        nc.vector.tensor_reduce(zs[:], z2[:], axis=mybir.AxisListType.X, op=ALU.add)
        nc.scalar.activation(zs[:], zs[:], AF.Ln)
        res = ctmp.tile([8, 1], F32, tag="res")
        nc.vector.tensor_add(res[:], zs[:], zm[:])
        nc.sync.dma_start(out_d[:], res[:])

    nc.compile()
    return nc


# ---------------------------------------------------------------- entry point

_CACHE = {}


def kernel(**inputs):
    T_ = np.asarray(inputs["x"]).shape[1]
    if T_ not in _CACHE:
        _CACHE[T_] = build_module(T_)
    nc = _CACHE[T_]
    shared, per_core = host_prep(inputs, T_)
    in_maps = [dict(shared, **pc) for pc in per_core]
    res = bass_utils.run_bass_kernel_spmd(
        nc, in_maps, core_ids=list(range(NCORE)),
        trace=bool(int(os.environ.get("KERNEL_TRACE", "0"))),
    )
    out = np.concatenate([res.results[c]["out"][:, 0] for c in range(NCORE)])
    kernel._last_results = res
    return out.astype(np.float32)

